# revision 24
# baseline (speedup 1.0000x reference)
"""Trainium2 Bass kernel for the top-k ranking metric layer.

Computes, for each of 8192 users with 1000 candidates (1 positive + 999
negatives, channel 1 of a softmax pair):
  - in_top_k:  1.0 if the positive item ranks in the top 10 (after masking
               duplicate candidates to -inf), else 0.0
  - ndcg:      ln(2)/ln(rank+2) * in_top_k
  - weights:   1.0 unless all 999 negatives are duplicates

Rank identity (stable descending argsort): rank(item 0) equals
count_j(masked[j] > masked[0]).  Computed with no masking pass at all:
  rank' = count_j((l[j] > l[0]) > d[j])       one fused DVE op per tile
  d0=0:  rank' == ref rank exactly ((l>l0) and not dup)
  d0=1:  ref rank = #non-dup >= 10 for any input without >990 dups per
         row, so in_top_k = 0 = (rank' < 10) > d[0] via one more fused
         compare; ndcg = 0 via the in_top_k factor.  Exact vs the
         reference except on ~2^-900-probability dup patterns.
The ACT pass that converts d to f32 accumulates sum(d) per user, giving
weights = (sum != 999) exactly for free.

Pipeline: dup halves stream on the ACT HWDGE ring while logits stream
on the sync ring (both rings drain from ~8us, and logits are not queued
behind the dup megabyte).  Logits tiles: halves for tile 0 (early DVE
start), wholes for 1-4, halves for 5-6 and 500+250+250 for tile 7 so
the tail tracks DMA completion semaphores at fine granularity.
1024 users per core.
"""

import numpy as np

_TRN_REPO = "/opt/trn_rl_repo"

NUM_CORES = 8
U = 8192                 # total users
ROW = 1000               # candidates per user
HALF = ROW // 2
QTR = ROW // 4
P = 128                  # SBUF partitions
U_CORE = U // NUM_CORES  # 1024 users per core
T = U_CORE // P          # 8 user-blocks per core
LN2 = float(np.log(2.0))
TOP_K = 10.0
DUP_ALL = 999.0          # sum(dup) value meaning "999 dups"

_NC = None


def _ensure_path():
    import sys
    try:
        import concourse  # noqa: F401
    except ImportError:
        sys.path.insert(0, _TRN_REPO)


def _build_nc():
    _ensure_path()
    from contextlib import ExitStack

    import concourse.tile as tile
    from concourse import bacc, mybir

    AF = mybir.ActivationFunctionType
    OP = mybir.AluOpType
    f32 = mybir.dt.float32
    i8 = mybir.dt.int8

    nc = bacc.Bacc(
        "TRN2", target_bir_lowering=False, debug=False, num_devices=NUM_CORES
    )
    # channel-1 logits only, de-interleaved on the host
    ld = nc.dram_tensor("logits", [T, P, ROW], f32, kind="ExternalInput").ap()
    # dup mask as int8, host-transposed to [P, T*ROW]
    dd = nc.dram_tensor("dup", [P, T * ROW], i8, kind="ExternalInput").ap()
    outd = nc.dram_tensor("out", [P, 3 * T], f32, kind="ExternalOutput").ap()

    with tile.TileContext(nc) as tc, ExitStack() as ctx:
        lg = ctx.enter_context(tc.tile_pool(name="lg", bufs=1))
        dp = ctx.enter_context(tc.tile_pool(name="dp", bufs=1))
        ps = ctx.enter_context(tc.tile_pool(name="ps", bufs=T))
        cm = ctx.enter_context(tc.tile_pool(name="cm", bufs=3))
        st = ctx.enter_context(tc.tile_pool(name="st", bufs=1))

        cnta = st.tile([P, T], f32, tag="cnta")  # rank partial, piece 1
        cntb = st.tile([P, T], f32, tag="cntb")  # rank partial, piece 2
        qex = st.tile([P, 1], f32, tag="qex")    # rank partial, piece 3 (t7)
        cnt = st.tile([P, T], f32, tag="cnt")    # rank of item 0, per user
        dsm = st.tile([P, T], f32, tag="dsm")    # sum(dup), per user
        outt = st.tile([P, 3 * T], f32, tag="outt")

        lts = [
            lg.tile([P, ROW], f32, name=f"lt{t}", tag=f"lt{t}") for t in range(T)
        ]
        H = T // 2
        dup_a = dp.tile([P, H * ROW], i8, name="dup_a", tag="dup_a")
        dup_b = dp.tile([P, H * ROW], i8, name="dup_b", tag="dup_b")
        # Dup halves on the ACT HWDGE ring, logits on the sync ring: both
        # rings drain from ~8us (faster DMA ramp), and the logits FIFO is
        # not queued behind the dup megabyte.
        nc.scalar.dma_start(dup_a[:], dd[:, 0 : H * ROW])
        nc.scalar.dma_start(dup_b[:], dd[:, H * ROW : T * ROW])
        nc.sync.dma_start(lts[0][:, 0:HALF], ld[0][:, 0:HALF])
        nc.sync.dma_start(lts[0][:, HALF:ROW], ld[0][:, HALF:ROW])
        for t in range(1, 5):
            nc.sync.dma_start(lts[t][:], ld[t])
        for t in (5, 6):
            nc.sync.dma_start(lts[t][:, 0:HALF], ld[t][:, 0:HALF])
            nc.sync.dma_start(lts[t][:, HALF:ROW], ld[t][:, HALF:ROW])
        nc.sync.dma_start(lts[7][:, 0:HALF], ld[7][:, 0:HALF])
        nc.sync.dma_start(lts[7][:, HALF : HALF + QTR], ld[7][:, HALF : HALF + QTR])
        nc.sync.dma_start(lts[7][:, HALF + QTR : ROW], ld[7][:, HALF + QTR : ROW])

        # Preload the Ln activation table during the DMA-bound phase so the
        # lazy ACT_TABLE_LOAD (~1.3us) doesn't land in the kernel tail.
        two = st.tile([P, 1], f32, tag="two")
        nc.vector.memset(two[:], 2.0)
        warm = st.tile([P, 1], f32, tag="warm")
        nc.scalar.activation(warm[:], two[:], AF.Ln, bias=two[:])

        nc.vector.memset(cntb[:, 1:5], 0.0)

        def dup_slice(t):
            half = dup_a if t < H else dup_b
            tt = t % H
            return half[:, tt * ROW : tt * ROW + ROW]

        for t in range(T):
            # df = d as f32 {0,1}; accum gives row-sum(dup) for weights
            df = ps.tile([P, ROW], f32, tag="df")
            nc.scalar.activation(
                df[:], dup_slice(t), AF.Copy, accum_out=dsm[:, t : t + 1]
            )

            l1 = lts[t][:]
            # cmp[j] = (l[j] > l[0]) > d[j], accumulated per arrival piece.
            # rank' is wrong for d[0]=1 users; the finish stage zeroes their
            # in_top_k via d0 (exact unless a row has >990 dups).
            cmp = cm.tile([P, ROW], f32, tag="cmp")

            def stt(lo, hi, acc):
                nc.vector.scalar_tensor_tensor(
                    cmp[:, lo:hi],
                    l1[:, lo:hi],
                    l1[:, 0:1],
                    df[:, lo:hi],
                    op0=OP.is_gt,
                    op1=OP.is_gt,
                    accum_out=acc,
                )

            if t == 0:
                stt(0, HALF, cnta[:, 0:1])
                stt(HALF, ROW, cntb[:, 0:1])
            elif t < 5:
                stt(0, ROW, cnta[:, t : t + 1])
            elif t < 7:
                stt(0, HALF, cnta[:, t : t + 1])
                stt(HALF, ROW, cntb[:, t : t + 1])
            else:
                # tiles 0-6 accums are complete: fold them now, in the DVE
                # gaps while tile 7's pieces arrive
                nc.vector.tensor_tensor(
                    cnt[:, 0:7], cnta[:, 0:7], cntb[:, 0:7], op=OP.add
                )
                stt(0, HALF, cnta[:, 7:8])
                stt(HALF, HALF + QTR, qex[:])
                stt(HALF + QTR, ROW, cntb[:, 7:8])

        # d0f[:, t] = d[0] of tile t (strided cast from the dup halves)
        d0f = st.tile([P, T], f32, tag="d0f")
        from concourse.ap import AP as _AP
        for hi, half in enumerate((dup_a, dup_b)):
            base = half[:]
            src = _AP(base.tensor, base.offset, [base.ap[0], [ROW, H]])
            nc.scalar.activation(d0f[:, hi * H : (hi + 1) * H], src, AF.Copy)

        # ---- finishing over [P, T] ----
        # weights = (sum(dup) != 999); dsm ready before the last logits tile
        nc.vector.tensor_scalar(
            outt[:, 2 * T : 3 * T], dsm[:], DUP_ALL, None, op0=OP.not_equal
        )
        nc.vector.scalar_tensor_tensor(
            cnt[:, 7:8], cnta[:, 7:8], qex[:], cntb[:, 7:8],
            op0=OP.add, op1=OP.add,
        )
        # in_top_k = (rank' < 10) and not d[0]
        nc.vector.scalar_tensor_tensor(
            outt[:, 0:T], cnt[:], TOP_K, d0f[:], op0=OP.is_lt, op1=OP.is_gt
        )
        # ndcg = ln2 / ln(rank + 2) * in_top_k
        lnp = st.tile([P, T], f32, tag="lnp")
        nc.scalar.activation(lnp[:], cnt[:], AF.Ln, bias=two[:])
        rcp = st.tile([P, T], f32, tag="rcp")
        nc.vector.reciprocal(rcp[:], lnp[:])
        nc.vector.scalar_tensor_tensor(
            outt[:, T : 2 * T],
            rcp[:],
            LN2,
            outt[:, 0:T],
            op0=OP.mult,
            op1=OP.mult,
        )
        nc.sync.dma_start(outd, outt[:])

    nc.compile()
    return nc


def _get_nc():
    global _NC
    if _NC is None:
        _NC = _build_nc()
    return _NC


def _shard_inputs(logits, dup_mask):
    # channel 1 only: [U*ROW, 1, 2] -> [NUM_CORES, T, P, ROW]
    l1 = np.ascontiguousarray(
        np.asarray(logits, dtype=np.float32).reshape(U * ROW, 2)[:, 1]
    ).reshape(NUM_CORES, T, P, ROW)
    # dup as int8, transposed to [NUM_CORES, P, T*ROW]
    dm = np.asarray(dup_mask, dtype=np.int32).astype(np.int8).reshape(
        NUM_CORES, T, P, ROW
    )
    d8 = np.ascontiguousarray(dm.transpose(0, 2, 1, 3)).reshape(
        NUM_CORES, P, T * ROW
    )
    return [{"logits": l1[c], "dup": d8[c]} for c in range(NUM_CORES)]


def _unshard_outputs(per_core_outs):
    # out[p, t] holds user t*128+p of the core (col-blocks: topk | ndcg | wts)
    full = np.stack(per_core_outs)  # [C, P, 3T]
    in_top_k = np.ascontiguousarray(
        full[:, :, 0:T].transpose(0, 2, 1).reshape(U), dtype=np.float32
    )
    ndcg = np.ascontiguousarray(
        full[:, :, T : 2 * T].transpose(0, 2, 1).reshape(U), dtype=np.float32
    )
    wts = np.ascontiguousarray(
        full[:, :, 2 * T : 3 * T].transpose(0, 2, 1).reshape(U), dtype=np.float32
    )
    return in_top_k, ndcg, wts


def _run(logits, dup_mask, trace=False, **kwargs):
    """Run on hardware; returns ((in_top_k, ndcg, weights), BassKernelResults)."""
    _ensure_path()
    from concourse.bass_utils import run_bass_kernel_spmd

    nc = _get_nc()
    in_maps = _shard_inputs(logits, dup_mask)
    res = run_bass_kernel_spmd(
        nc, in_maps, core_ids=list(range(NUM_CORES)), trace=trace, **kwargs
    )
    outs = [res.results[c]["out"] for c in range(NUM_CORES)]
    return _unshard_outputs(outs), res


def kernel(logits, dup_mask):
    (in_top_k, ndcg, wts), _ = _run(logits, dup_mask)
    return in_top_k, ndcg, wts


# revision 25
# speedup vs baseline: 1.1560x; 1.1560x over previous
"""Trainium2 Bass kernel for the top-k ranking metric layer.

Computes, for each of 8192 users with 1000 candidates (1 positive + 999
negatives, channel 1 of a softmax pair):
  - in_top_k:  1.0 if the positive item ranks in the top 10 (after masking
               duplicate candidates to -inf), else 0.0
  - ndcg:      ln(2)/ln(rank+2) * in_top_k
  - weights:   1.0 unless all 999 negatives are duplicates

Rank identity (stable descending argsort): rank(item 0) equals
count_j(masked[j] > masked[0]).  Computed with no masking pass at all:
  rank' = count_j((l[j] > l[0]) > d[j])       one fused DVE op per tile
  d0=0:  rank' == ref rank exactly ((l>l0) and not dup)
  d0=1:  ref rank = #non-dup >= 10 for any input without >990 dups per
         row, so in_top_k = 0 = (rank' < 10) > d[0] via one more fused
         compare; ndcg = 0 via the in_top_k factor.  Exact vs the
         reference except on ~2^-900-probability dup patterns.
The ACT pass that converts d to f32 accumulates sum(d) per user, giving
weights = (sum != 999) exactly for free.

Pipeline: dup halves stream on the ACT HWDGE ring while logits stream
on the sync ring (both rings drain from ~8us, and logits are not queued
behind the dup megabyte).  Logits tiles: halves for tile 0 (early DVE
start), wholes for 1-4, halves for 5-6 and 500+250+250 for tile 7 so
the tail tracks DMA completion semaphores at fine granularity.
1024 users per core.
"""

import numpy as np

_TRN_REPO = "/opt/trn_rl_repo"

NUM_CORES = 8
U = 8192                 # total users
ROW = 1000               # candidates per user
HALF = ROW // 2
QTR = ROW // 4
P = 128                  # SBUF partitions
U_CORE = U // NUM_CORES  # 1024 users per core
T = U_CORE // P          # 8 user-blocks per core
LN2 = float(np.log(2.0))
TOP_K = 10.0
DUP_ALL = 999.0          # sum(dup) value meaning "999 dups"

_NC = None


def _ensure_path():
    import sys
    try:
        import concourse  # noqa: F401
    except ImportError:
        sys.path.insert(0, _TRN_REPO)


def _build_nc():
    _ensure_path()
    from contextlib import ExitStack

    import concourse.tile as tile
    from concourse import bacc, mybir

    AF = mybir.ActivationFunctionType
    OP = mybir.AluOpType
    f32 = mybir.dt.float32
    i8 = mybir.dt.int8

    nc = bacc.Bacc(
        "TRN2", target_bir_lowering=False, debug=False, num_devices=NUM_CORES
    )
    # channel-1 logits only, de-interleaved on the host
    ld = nc.dram_tensor("logits", [T, P, ROW], f32, kind="ExternalInput").ap()
    # dup mask as int8, host-transposed to [P, T*ROW]
    dd = nc.dram_tensor("dup", [P, T * ROW], i8, kind="ExternalInput").ap()
    outd = nc.dram_tensor("out", [P, 3 * T], f32, kind="ExternalOutput").ap()

    with tile.TileContext(nc) as tc, ExitStack() as ctx:
        lg = ctx.enter_context(tc.tile_pool(name="lg", bufs=1))
        dp = ctx.enter_context(tc.tile_pool(name="dp", bufs=1))
        ps = ctx.enter_context(tc.tile_pool(name="ps", bufs=T))
        cm = ctx.enter_context(tc.tile_pool(name="cm", bufs=3))
        st = ctx.enter_context(tc.tile_pool(name="st", bufs=1))

        cnta = st.tile([P, T], f32, tag="cnta")  # rank partial, piece 1
        cntb = st.tile([P, T], f32, tag="cntb")  # rank partial, piece 2
        qex = st.tile([P, 1], f32, tag="qex")    # rank partial, piece 3 (t7)
        cnt = st.tile([P, T], f32, tag="cnt")    # rank of item 0, per user
        dsm = st.tile([P, T], f32, tag="dsm")    # sum(dup), per user
        outt = st.tile([P, 3 * T], f32, tag="outt")

        lts = [
            lg.tile([P, ROW], f32, name=f"lt{t}", tag=f"lt{t}") for t in range(T)
        ]
        H = T // 2
        dup_a = dp.tile([P, H * ROW], i8, name="dup_a", tag="dup_a")
        dup_b = dp.tile([P, H * ROW], i8, name="dup_b", tag="dup_b")
        # Dup halves on the ACT HWDGE ring, logits on the sync ring: both
        # rings drain from ~8us (faster DMA ramp), and the logits FIFO is
        # not queued behind the dup megabyte.
        nc.scalar.dma_start(dup_a[:], dd[:, 0 : H * ROW])
        nc.scalar.dma_start(dup_b[:], dd[:, H * ROW : T * ROW])
        nc.sync.dma_start(lts[0][:, 0:HALF], ld[0][:, 0:HALF])
        nc.sync.dma_start(lts[0][:, HALF:ROW], ld[0][:, HALF:ROW])
        for t in range(1, 5):
            nc.sync.dma_start(lts[t][:], ld[t])
        for t in (5, 6):
            nc.sync.dma_start(lts[t][:, 0:HALF], ld[t][:, 0:HALF])
            nc.sync.dma_start(lts[t][:, HALF:ROW], ld[t][:, HALF:ROW])
        nc.sync.dma_start(lts[7][:, 0:HALF], ld[7][:, 0:HALF])
        nc.sync.dma_start(lts[7][:, HALF : HALF + QTR], ld[7][:, HALF : HALF + QTR])
        nc.sync.dma_start(lts[7][:, HALF + QTR : ROW], ld[7][:, HALF + QTR : ROW])

        # Preload the Ln activation table during the DMA-bound phase so the
        # lazy ACT_TABLE_LOAD (~1.3us) doesn't land in the kernel tail.
        two = st.tile([P, 1], f32, tag="two")
        nc.vector.memset(two[:], 2.0)
        warm = st.tile([P, 1], f32, tag="warm")
        nc.scalar.activation(warm[:], two[:], AF.Ln, bias=two[:])

        def dup_slice(t):
            half = dup_a if t < H else dup_b
            tt = t % H
            return half[:, tt * ROW : tt * ROW + ROW]

        for t in range(T):
            # df = d as f32 {0,1}; accum gives row-sum(dup) for weights
            df = ps.tile([P, ROW], f32, tag="df")
            nc.scalar.activation(
                df[:], dup_slice(t), AF.Copy, accum_out=dsm[:, t : t + 1]
            )

            l1 = lts[t][:]
            # cmp[j] = (l[j] > l[0]) > d[j], accumulated per arrival piece.
            # rank' is wrong for d[0]=1 users; the finish stage zeroes their
            # in_top_k via d0 (exact unless a row has >990 dups).
            cmp = cm.tile([P, ROW], f32, tag="cmp")

            def stt(lo, hi, acc):
                nc.vector.scalar_tensor_tensor(
                    cmp[:, lo:hi],
                    l1[:, lo:hi],
                    l1[:, 0:1],
                    df[:, lo:hi],
                    op0=OP.is_gt,
                    op1=OP.is_gt,
                    accum_out=acc,
                )

            if t == 0:
                stt(0, HALF, cnta[:, 0:1])
                stt(HALF, ROW, cntb[:, 0:1])
            elif t < 5:
                stt(0, ROW, cnta[:, t : t + 1])
            elif t < 7:
                stt(0, HALF, cnta[:, t : t + 1])
                stt(HALF, ROW, cntb[:, t : t + 1])
            else:
                stt(0, HALF, cnta[:, 7:8])
                stt(HALF, HALF + QTR, qex[:])
                stt(HALF + QTR, ROW, cntb[:, 7:8])

        nc.vector.memset(cntb[:, 1:5], 0.0)

        # d0f[:, t] = d[0] of tile t (strided cast from the dup halves)
        d0f = st.tile([P, T], f32, tag="d0f")
        from concourse.ap import AP as _AP
        for hi, half in enumerate((dup_a, dup_b)):
            base = half[:]
            src = _AP(base.tensor, base.offset, [base.ap[0], [ROW, H]])
            nc.scalar.activation(d0f[:, hi * H : (hi + 1) * H], src, AF.Copy)

        # ---- finishing over [P, T] ----
        # weights = (sum(dup) != 999); dsm ready before the last logits tile
        nc.vector.tensor_scalar(
            outt[:, 2 * T : 3 * T], dsm[:], DUP_ALL, None, op0=OP.not_equal
        )
        nc.vector.tensor_tensor(cnt[:], cnta[:], cntb[:], op=OP.add)
        nc.vector.tensor_tensor(cnt[:, 7:8], cnt[:, 7:8], qex[:], op=OP.add)
        # in_top_k = (rank' < 10) and not d[0]
        nc.vector.scalar_tensor_tensor(
            outt[:, 0:T], cnt[:], TOP_K, d0f[:], op0=OP.is_lt, op1=OP.is_gt
        )
        # ndcg = ln2 / ln(rank + 2) * in_top_k
        lnp = st.tile([P, T], f32, tag="lnp")
        nc.scalar.activation(lnp[:], cnt[:], AF.Ln, bias=two[:])
        rcp = st.tile([P, T], f32, tag="rcp")
        nc.vector.reciprocal(rcp[:], lnp[:])
        nc.vector.scalar_tensor_tensor(
            outt[:, T : 2 * T],
            rcp[:],
            LN2,
            outt[:, 0:T],
            op0=OP.mult,
            op1=OP.mult,
        )
        nc.sync.dma_start(outd, outt[:])

    nc.compile()
    return nc


def _get_nc():
    global _NC
    if _NC is None:
        _NC = _build_nc()
    return _NC


def _shard_inputs(logits, dup_mask):
    # channel 1 only: [U*ROW, 1, 2] -> [NUM_CORES, T, P, ROW]
    l1 = np.ascontiguousarray(
        np.asarray(logits, dtype=np.float32).reshape(U * ROW, 2)[:, 1]
    ).reshape(NUM_CORES, T, P, ROW)
    # dup as int8, transposed to [NUM_CORES, P, T*ROW]
    dm = np.asarray(dup_mask, dtype=np.int32).astype(np.int8).reshape(
        NUM_CORES, T, P, ROW
    )
    d8 = np.ascontiguousarray(dm.transpose(0, 2, 1, 3)).reshape(
        NUM_CORES, P, T * ROW
    )
    return [{"logits": l1[c], "dup": d8[c]} for c in range(NUM_CORES)]


def _unshard_outputs(per_core_outs):
    # out[p, t] holds user t*128+p of the core (col-blocks: topk | ndcg | wts)
    full = np.stack(per_core_outs)  # [C, P, 3T]
    in_top_k = np.ascontiguousarray(
        full[:, :, 0:T].transpose(0, 2, 1).reshape(U), dtype=np.float32
    )
    ndcg = np.ascontiguousarray(
        full[:, :, T : 2 * T].transpose(0, 2, 1).reshape(U), dtype=np.float32
    )
    wts = np.ascontiguousarray(
        full[:, :, 2 * T : 3 * T].transpose(0, 2, 1).reshape(U), dtype=np.float32
    )
    return in_top_k, ndcg, wts


def _run(logits, dup_mask, trace=False, **kwargs):
    """Run on hardware; returns ((in_top_k, ndcg, weights), BassKernelResults)."""
    _ensure_path()
    from concourse.bass_utils import run_bass_kernel_spmd

    nc = _get_nc()
    in_maps = _shard_inputs(logits, dup_mask)
    res = run_bass_kernel_spmd(
        nc, in_maps, core_ids=list(range(NUM_CORES)), trace=trace, **kwargs
    )
    outs = [res.results[c]["out"] for c in range(NUM_CORES)]
    return _unshard_outputs(outs), res


def kernel(logits, dup_mask):
    (in_top_k, ndcg, wts), _ = _run(logits, dup_mask)
    return in_top_k, ndcg, wts


# revision 26
# speedup vs baseline: 1.1617x; 1.0050x over previous
"""Trainium2 Bass kernel for the top-k ranking metric layer.

Computes, for each of 8192 users with 1000 candidates (1 positive + 999
negatives, channel 1 of a softmax pair):
  - in_top_k:  1.0 if the positive item ranks in the top 10 (after masking
               duplicate candidates to -inf), else 0.0
  - ndcg:      ln(2)/ln(rank+2) * in_top_k
  - weights:   1.0 unless all 999 negatives are duplicates

Rank identity (stable descending argsort): rank(item 0) equals
count_j(masked[j] > masked[0]).  Computed with no masking pass at all:
  rank' = count_j((l[j] > l[0]) > d[j])       one fused DVE op per tile
  d0=0:  rank' == ref rank exactly ((l>l0) and not dup)
  d0=1:  ref rank = #non-dup >= 10 for any input without >990 dups per
         row, so in_top_k = 0 = (rank' < 10) > d[0] via one more fused
         compare; ndcg = 0 via the in_top_k factor.  Exact vs the
         reference except on ~2^-900-probability dup patterns.
The ACT pass that converts d to f32 accumulates sum(d) per user, giving
weights = (sum != 999) exactly for free.

Pipeline: dup halves stream on the ACT HWDGE ring while logits stream
on the sync ring (both rings drain from ~8us, and logits are not queued
behind the dup megabyte).  Logits tiles: halves for tile 0 (early DVE
start), wholes for 1-4, halves for 5-6 and 500+250+250 for tile 7 so
the tail tracks DMA completion semaphores at fine granularity.
1024 users per core.
"""

import numpy as np

_TRN_REPO = "/opt/trn_rl_repo"

NUM_CORES = 8
U = 8192                 # total users
ROW = 1000               # candidates per user
HALF = ROW // 2
QTR = ROW // 4
P = 128                  # SBUF partitions
U_CORE = U // NUM_CORES  # 1024 users per core
T = U_CORE // P          # 8 user-blocks per core
LN2 = float(np.log(2.0))
TOP_K = 10.0
DUP_ALL = 999.0          # sum(dup) value meaning "999 dups"

_NC = None


def _ensure_path():
    import sys
    try:
        import concourse  # noqa: F401
    except ImportError:
        sys.path.insert(0, _TRN_REPO)


def _build_nc():
    _ensure_path()
    from contextlib import ExitStack

    import concourse.tile as tile
    from concourse import bacc, mybir

    AF = mybir.ActivationFunctionType
    OP = mybir.AluOpType
    f32 = mybir.dt.float32
    i8 = mybir.dt.int8

    nc = bacc.Bacc(
        "TRN2", target_bir_lowering=False, debug=False, num_devices=NUM_CORES
    )
    # channel-1 logits only, de-interleaved on the host
    ld = nc.dram_tensor("logits", [T, P, ROW], f32, kind="ExternalInput").ap()
    # dup mask as int8, host-transposed to [P, T*ROW]
    dd = nc.dram_tensor("dup", [P, T * ROW], i8, kind="ExternalInput").ap()
    outd = nc.dram_tensor("out", [P, 3 * T], f32, kind="ExternalOutput").ap()

    with tile.TileContext(nc) as tc, ExitStack() as ctx:
        lg = ctx.enter_context(tc.tile_pool(name="lg", bufs=1))
        dp = ctx.enter_context(tc.tile_pool(name="dp", bufs=1))
        ps = ctx.enter_context(tc.tile_pool(name="ps", bufs=T))
        cm = ctx.enter_context(tc.tile_pool(name="cm", bufs=3))
        st = ctx.enter_context(tc.tile_pool(name="st", bufs=1))

        cnta = st.tile([P, T], f32, tag="cnta")  # rank partial, piece 1
        cntb = st.tile([P, T], f32, tag="cntb")  # rank partial, piece 2
        qex = st.tile([P, 1], f32, tag="qex")    # rank partial, piece 3 (t7)
        cnt = st.tile([P, T], f32, tag="cnt")    # rank of item 0, per user
        dsm = st.tile([P, T], f32, tag="dsm")    # sum(dup), per user
        outt = st.tile([P, 3 * T], f32, tag="outt")

        lts = [
            lg.tile([P, ROW], f32, name=f"lt{t}", tag=f"lt{t}") for t in range(T)
        ]
        H = T // 2
        dup_a = dp.tile([P, H * ROW], i8, name="dup_a", tag="dup_a")
        dup_b = dp.tile([P, H * ROW], i8, name="dup_b", tag="dup_b")
        # Dup halves on the ACT HWDGE ring, logits on the sync ring: both
        # rings drain from ~8us (faster DMA ramp), and the logits FIFO is
        # not queued behind the dup megabyte.
        nc.scalar.dma_start(dup_a[:], dd[:, 0 : H * ROW])
        nc.scalar.dma_start(dup_b[:], dd[:, H * ROW : T * ROW])
        nc.sync.dma_start(lts[0][:, 0:HALF], ld[0][:, 0:HALF])
        nc.sync.dma_start(lts[0][:, HALF:ROW], ld[0][:, HALF:ROW])
        for t in range(1, 5):
            nc.sync.dma_start(lts[t][:], ld[t])
        for t in (5, 6):
            nc.sync.dma_start(lts[t][:, 0:HALF], ld[t][:, 0:HALF])
            nc.sync.dma_start(lts[t][:, HALF:ROW], ld[t][:, HALF:ROW])
        nc.sync.dma_start(lts[7][:, 0:HALF], ld[7][:, 0:HALF])
        nc.sync.dma_start(lts[7][:, HALF : HALF + QTR], ld[7][:, HALF : HALF + QTR])
        nc.sync.dma_start(lts[7][:, HALF + QTR : ROW], ld[7][:, HALF + QTR : ROW])

        # Preload the Ln activation table during the DMA-bound phase so the
        # lazy ACT_TABLE_LOAD (~1.3us) doesn't land in the kernel tail.
        two = st.tile([P, 1], f32, tag="two")
        nc.vector.memset(two[:], 2.0)
        warm = st.tile([P, 1], f32, tag="warm")
        nc.scalar.activation(warm[:], two[:], AF.Ln, bias=two[:])

        nc.vector.memset(cntb[:, 1:5], 0.0)

        def dup_slice(t):
            half = dup_a if t < H else dup_b
            tt = t % H
            return half[:, tt * ROW : tt * ROW + ROW]

        for t in range(T):
            # df = d as f32 {0,1}; accum gives row-sum(dup) for weights
            df = ps.tile([P, ROW], f32, tag="df")
            nc.scalar.activation(
                df[:], dup_slice(t), AF.Copy, accum_out=dsm[:, t : t + 1]
            )

            l1 = lts[t][:]
            # cmp[j] = (l[j] > l[0]) > d[j], accumulated per arrival piece.
            # rank' is wrong for d[0]=1 users; the finish stage zeroes their
            # in_top_k via d0 (exact unless a row has >990 dups).
            cmp = cm.tile([P, ROW], f32, tag="cmp")

            def stt(lo, hi, acc):
                nc.vector.scalar_tensor_tensor(
                    cmp[:, lo:hi],
                    l1[:, lo:hi],
                    l1[:, 0:1],
                    df[:, lo:hi],
                    op0=OP.is_gt,
                    op1=OP.is_gt,
                    accum_out=acc,
                )

            if t == 0:
                stt(0, HALF, cnta[:, 0:1])
                stt(HALF, ROW, cntb[:, 0:1])
            elif t < 5:
                stt(0, ROW, cnta[:, t : t + 1])
            elif t < 7:
                stt(0, HALF, cnta[:, t : t + 1])
                stt(HALF, ROW, cntb[:, t : t + 1])
            else:
                # tiles 0-6 accums are complete: fold them now, in the DVE
                # gaps while tile 7's pieces arrive
                nc.vector.tensor_tensor(
                    cnt[:, 0:7], cnta[:, 0:7], cntb[:, 0:7], op=OP.add
                )
                stt(0, HALF, cnta[:, 7:8])
                stt(HALF, HALF + QTR, qex[:])
                stt(HALF + QTR, ROW, cntb[:, 7:8])

        # d0f[:, t] = d[0] of tile t (strided cast from the dup halves)
        d0f = st.tile([P, T], f32, tag="d0f")
        from concourse.ap import AP as _AP
        for hi, half in enumerate((dup_a, dup_b)):
            base = half[:]
            src = _AP(base.tensor, base.offset, [base.ap[0], [ROW, H]])
            nc.scalar.activation(d0f[:, hi * H : (hi + 1) * H], src, AF.Copy)

        # ---- finishing over [P, T] ----
        # weights = (sum(dup) != 999); dsm ready before the last logits tile
        nc.vector.tensor_scalar(
            outt[:, 2 * T : 3 * T], dsm[:], DUP_ALL, None, op0=OP.not_equal
        )
        nc.vector.scalar_tensor_tensor(
            cnt[:, 7:8], cnta[:, 7:8], qex[:], cntb[:, 7:8],
            op0=OP.add, op1=OP.add,
        )
        # in_top_k = (rank' < 10) and not d[0]
        nc.vector.scalar_tensor_tensor(
            outt[:, 0:T], cnt[:], TOP_K, d0f[:], op0=OP.is_lt, op1=OP.is_gt
        )
        # ndcg = ln2 / ln(rank + 2) * in_top_k
        lnp = st.tile([P, T], f32, tag="lnp")
        nc.scalar.activation(lnp[:], cnt[:], AF.Ln, bias=two[:])
        rcp = st.tile([P, T], f32, tag="rcp")
        nc.vector.reciprocal(rcp[:], lnp[:])
        nc.vector.scalar_tensor_tensor(
            outt[:, T : 2 * T],
            rcp[:],
            LN2,
            outt[:, 0:T],
            op0=OP.mult,
            op1=OP.mult,
        )
        nc.sync.dma_start(outd, outt[:])

    nc.compile()
    return nc


def _get_nc():
    global _NC
    if _NC is None:
        _NC = _build_nc()
    return _NC


def _shard_inputs(logits, dup_mask):
    # channel 1 only: [U*ROW, 1, 2] -> [NUM_CORES, T, P, ROW]
    l1 = np.ascontiguousarray(
        np.asarray(logits, dtype=np.float32).reshape(U * ROW, 2)[:, 1]
    ).reshape(NUM_CORES, T, P, ROW)
    # dup as int8, transposed to [NUM_CORES, P, T*ROW]
    dm = np.asarray(dup_mask, dtype=np.int32).astype(np.int8).reshape(
        NUM_CORES, T, P, ROW
    )
    d8 = np.ascontiguousarray(dm.transpose(0, 2, 1, 3)).reshape(
        NUM_CORES, P, T * ROW
    )
    return [{"logits": l1[c], "dup": d8[c]} for c in range(NUM_CORES)]


def _unshard_outputs(per_core_outs):
    # out[p, t] holds user t*128+p of the core (col-blocks: topk | ndcg | wts)
    full = np.stack(per_core_outs)  # [C, P, 3T]
    in_top_k = np.ascontiguousarray(
        full[:, :, 0:T].transpose(0, 2, 1).reshape(U), dtype=np.float32
    )
    ndcg = np.ascontiguousarray(
        full[:, :, T : 2 * T].transpose(0, 2, 1).reshape(U), dtype=np.float32
    )
    wts = np.ascontiguousarray(
        full[:, :, 2 * T : 3 * T].transpose(0, 2, 1).reshape(U), dtype=np.float32
    )
    return in_top_k, ndcg, wts


def _run(logits, dup_mask, trace=False, **kwargs):
    """Run on hardware; returns ((in_top_k, ndcg, weights), BassKernelResults)."""
    _ensure_path()
    from concourse.bass_utils import run_bass_kernel_spmd

    nc = _get_nc()
    in_maps = _shard_inputs(logits, dup_mask)
    res = run_bass_kernel_spmd(
        nc, in_maps, core_ids=list(range(NUM_CORES)), trace=trace, **kwargs
    )
    outs = [res.results[c]["out"] for c in range(NUM_CORES)]
    return _unshard_outputs(outs), res


def kernel(logits, dup_mask):
    (in_top_k, ndcg, wts), _ = _run(logits, dup_mask)
    return in_top_k, ndcg, wts
